# revision 32
# baseline (speedup 1.0000x reference)
"""RNN-T JointNetwork Trainium2 kernel.

logits[b,t,u,v] = sum_j W_out[v,j] * tanh(f[b,t,j] + g[b,u,j]) + b_out[v]
  f = enc_out @ W_enc.T   [B,T,640]
  g = pred_out @ W_pred.T [B,U,640]

Sharding: data-parallel over B=8 across the 8 NeuronCores (1 batch/core).

v3 design notes (vs the int8 baseline at 342 us):
  The baseline saturated all three compute engines (~90% each): the
  per-row int8 absmax/scale machinery cost VectorE+ScalarE as much time
  as the matmul itself, and tanh read fT from PSUM in f32 (ScalarE reads
  f32 at half rate). This version outputs fp16 (skips absmax entirely)
  and feeds tanh from fp16 SBUF, leaving the PE as the only near-
  saturated engine (bf16/fp16 matmul roofline ~277 us).

  phase 1: fT[j,t] (=f.T) accumulated in PSUM then drained to SBUF fp16;
           gT[j,u] drained f32 (used as tanh bias operand). Inputs
           arrive as per-j-chunk DMAs so each j group starts on its
           chunk's arrival; cold warmup matmuls bridge the preamble->
           first-arrival window to keep the PE HAM clock-gate busy.
  phase 2: comb[j,uu,t] = tanh(fT + gT[:,u]) via ScalarE activation with
           per-partition bias, fp16 out; one tile per u-PAIR so a matmul
           can stream 512 moving columns (2 u x 256 t).
  phase 3: vocab-on-partition orientation: for each (u-pair block, vt):
           out_ps[v,(uu,t)] += wout[j,vt-chunk].T @ comb[j] with j outer,
           pair inner -> each weight load feeds up to 4 N=512 matmuls
           (vs 1:2 in the t-orientation), up to 4 psum banks x 2
           generations = 8 banks. Block sizes ramp 1,1,2,3,4... so the
           first matmuls start after only 10 activations, and ramp back
           down at the end to shrink the final serial drain tail.
  phase 4: VectorE drains each bank with ONE fused op: fp16 out =
           psum_f32 + bias_v (tensor_scalar add, per-partition bias),
           then DMA [128v, 2u x 256t] -> HBM [vt,v,u,t], 1 KiB contiguous
           per partition (all output DMAs on the otherwise-idle sync
           queue). Host transposes to [T,U,V] and upcasts.

  All matmul operands are fp16 (PE upconverts to e10m11 internally, so
  fp16 is strictly more accurate than bf16 at identical speed). fp8
  (DoubleRow, ~1.5x) was evaluated and rejected: measured e4m3
  quantization noise is 3.3% L2 vs the 2e-2 gate, and every
  residual-correction scheme costs >= the bf16 slot count.
"""

import os
import sys

for _p in ("/opt/trn_rl_repo",):
    if _p not in sys.path:
        sys.path.insert(0, _p)

import numpy as np


def _enable_jax_compile_cache():
    """Persistent XLA executable cache: skips the per-process
    HLO->walrus->NEFF compile of the wrapped bass_exec call (~1s) when
    warm. jax is pre-imported by the site hook, so set config directly;
    cache errors are non-fatal to jax."""
    try:
        cc = os.path.expanduser("~/.cache/jax_bass_cc")
        os.makedirs(cc, exist_ok=True)
        import jax

        jax.config.update("jax_compilation_cache_dir", cc)
        jax.config.update("jax_persistent_cache_min_compile_time_secs", 0.0)
    except Exception:
        pass

B, T, U = 8, 256, 64
D_ENC, D_PRED, D_JOINT, VOCAB = 512, 512, 640, 1024
KE = D_ENC // 128   # 4 contraction chunks for enc/pred matmuls
KJ = D_JOINT // 128  # 5 contraction chunks for the vocab matmul
N_CORES = 8
VT = VOCAB // 128   # 8 vocab partition tiles
UB = 8              # u's per ublock
NP = UB // 2        # u-pairs per ublock

_compiled = None


def _build():
    import concourse.bacc as bacc
    import concourse.bass as bass
    import concourse.mybir as mybir
    import concourse.tile as tile

    f32 = mybir.dt.float32
    f16 = mybir.dt.float16
    PSUM = bass.MemorySpace.PSUM
    tanh = mybir.ActivationFunctionType.Tanh
    add = mybir.AluOpType.add

    nc = bacc.Bacc(
        "TRN2",
        target_bir_lowering=False,
        debug=False,
        enable_asserts=False,
    )

    enc_d = nc.dram_tensor("enc", [128, KE, T], f16, kind="ExternalInput")
    pred_d = nc.dram_tensor("pred", [128, KE, U], f16, kind="ExternalInput")
    wenc_d = nc.dram_tensor("wenc", [128, KJ, KE, 128], f16, kind="ExternalInput")
    wpred_d = nc.dram_tensor("wpred", [128, KJ, KE, 128], f16, kind="ExternalInput")
    wout_d = nc.dram_tensor("wout", [128, VT, KJ, 128], f16, kind="ExternalInput")
    bias_d = nc.dram_tensor("bias", [128, VT], f32, kind="ExternalInput")
    out_d = nc.dram_tensor("out", [VT, 128, U, T], f16, kind="ExternalOutput")

    out_qs = None  # round-robin DMA trigger engines for output tiles

    with tile.TileContext(nc) as tc:
        with (
            tc.tile_pool(name="const", bufs=1) as const,
            tc.tile_pool(name="comb", bufs=2) as comb_pool,
            tc.tile_pool(name="stage", bufs=3) as stage_pool,
        ):
            # Trigger the Tanh ACT table load before any data arrives.
            warm = const.tile([1, 8], f32)
            warm2 = const.tile([1, 8], f32)
            nc.vector.memset(warm[:], 0.0)
            nc.scalar.activation(warm2[:], warm[:], tanh)

            enc_sb = const.tile([128, KE, T], f16)
            pred_sb = const.tile([128, KE, U], f16)
            wenc_sb = const.tile([128, KJ, KE, 128], f16)
            wpred_sb = const.tile([128, KJ, KE, 128], f16)
            wout_sb = const.tile([128, VT, KJ, 128], f16)
            bias_sb = const.tile([128, VT], f32)
            fT_sb = const.tile([128, KJ, T], f16)
            gT_sb = const.tile([128, KJ, U], f32)

            # PE warmup: dummy matmuls on zeroed data while input DMAs are
            # in flight, so HAM un-throttles before the real matmuls start.
            wz = const.tile([128, 512], f16)
            nc.gpsimd.memset(wz[:], 0.0)

            # Input DMA triggers on sync+gpsimd only (scalar's FIFO must
            # stay clear for the tanh stream), chunked so each phase-1
            # j-group starts on its chunk's arrival; wout chunks trickle
            # in behind the critical wenc chunks in vt consumption order.
            # NOTE: queue assignment is bandwidth-ordered, not just
            # latency-ordered — all queues share HBM read bandwidth, so
            # wout must sit BEHIND the critical wenc/wpred chunks (moving
            # it to an idle queue makes it contend and delays phase 1).
            nc.sync.dma_start(enc_sb[:, 0:1], enc_d[:, 0:1])
            nc.gpsimd.dma_start(pred_sb[:], pred_d[:])
            nc.sync.dma_start(wenc_sb[:, 0], wenc_d[:, 0])
            nc.sync.dma_start(enc_sb[:, 1:], enc_d[:, 1:])
            for j in range(KJ):
                nc.gpsimd.dma_start(wpred_sb[:, j], wpred_d[:, j])
                if j > 0:
                    nc.sync.dma_start(wenc_sb[:, j], wenc_d[:, j])
            nc.gpsimd.dma_start(bias_sb[:], bias_d[:])
            for vt in range(VT):
                nc.gpsimd.dma_start(wout_sb[:, vt], wout_d[:, vt])

            # Cold N=256 warmup matmuls bridge the ~2 us from preamble
            # end to the first input chunk arrival, keeping HAM busy.
            with tc.tile_pool(name="psw", bufs=1, space=PSUM) as psw:
                pw = psw.tile([128, 512], f32)
                for i in range(5):
                    nc.tensor.matmul(pw[:, 0:256], wz[:, :128], wz[:, 0:256],
                                     start=True, stop=True)

            # phase 1: per j-chunk, gT (small, drained while fT matmuls
            # run) then fT. (A gT/fT emission weave matched to chunk
            # arrivals was tried and measured neutral — the head is
            # DMA-arrival-bound, not FIFO-ordering-bound.)
            with tc.tile_pool(name="psg", bufs=2, space=PSUM) as psg:
                for j in range(KJ):
                    psp = psg.tile([128, U], f32, tag="psp")
                    for k in range(KE):
                        nc.tensor.matmul(
                            psp[:],
                            wpred_sb[:, j, k, :],
                            pred_sb[:, k, :],
                            start=(k == 0),
                            stop=(k == KE - 1),
                        )
                    nc.vector.tensor_copy(gT_sb[:, j, :], psp[:])
                    psf = psg.tile([128, T], f32, tag="psf")
                    for k in range(KE):
                        nc.tensor.matmul(
                            psf[:],
                            wenc_sb[:, j, k, :],
                            enc_sb[:, k, :],
                            start=(k == 0),
                            stop=(k == KE - 1),
                        )
                    nc.vector.tensor_copy(fT_sb[:, j, :], psf[:])

            def make_comb(pairs):
                """Emit the tanh activations for the given u-pair indices;
                returns one tile [128, KJ, 2, T] fp16 per pair."""
                tiles = []
                for slot, q in enumerate(pairs):
                    cp = comb_pool.tile([128, KJ, 2, T], f16, tag=f"comb{slot}",
                                        name=f"comb{slot}_{q}")
                    for uu in range(2):
                        u = q * 2 + uu
                        for j in range(KJ):
                            nc.scalar.activation(
                                cp[:, j, uu, :],
                                fT_sb[:, j, :],
                                tanh,
                                bias=gT_sb[:, j, u:u + 1],
                            )
                    tiles.append(cp)
                return tiles

            # u-pair blocks: the leading ramp is sized so each block's
            # tanh activations fit inside the previous block's matmul
            # span; small trailing blocks shrink the serial drain tail
            # after the last matmul.
            blocks = [[0], [1], [2, 3], [4, 5, 6]] + [
                [q, q + 1, q + 2, q + 3] for q in range(7, 27, 4)
            ] + [[27, 28], [29, 30], [31]]

            with tc.tile_pool(name="pso", bufs=2, space=PSUM) as pso:
                combs = make_comb(blocks[0])
                for bi, prs in enumerate(blocks):
                    last = bi == len(blocks) - 1
                    combs_next = None if last else make_comb(blocks[bi + 1])
                    for vt in range(VT):
                        pss = [
                            pso.tile([128, 512], f32, tag=f"ps{slot}",
                                     name=f"ps{slot}_{bi}_{vt}")
                            for slot in range(len(prs))
                        ]
                        for j in range(KJ):
                            w = wout_sb[:, vt, j, :]
                            for slot in range(len(prs)):
                                nc.tensor.matmul(
                                    pss[slot][:],
                                    w,
                                    combs[slot][:, j, :, :],
                                    start=(j == 0),
                                    stop=(j == KJ - 1),
                                )
                        # One staging tile and ONE output DMA per (block,
                        # vt) group: fewer sync-queue triggers/completion
                        # semaphores (the u-range is contiguous in HBM).
                        npr = len(prs)
                        stg = stage_pool.tile([128, npr * 512], f16,
                                              tag=f"stg{npr}",
                                              name=f"stg{npr}_{bi}_{vt}")
                        for slot in range(npr):
                            nc.vector.tensor_scalar(
                                stg[:, slot * 512:(slot + 1) * 512],
                                pss[slot][:], bias_sb[:, vt:vt + 1],
                                None, op0=add,
                            )
                        u0 = 2 * prs[0]
                        nc.sync.dma_start(
                            out_d[vt, :, u0:u0 + 2 * npr, :], stg[:])
                    combs = combs_next

    nc.compile()
    return nc


def _get_compiled():
    global _compiled
    if _compiled is None:
        _compiled = _build()
    return _compiled


def _prep_inputs(enc_out, pred_out, W_enc, W_pred, W_out, b_out):
    f16 = np.float16
    enc_out = np.asarray(enc_out, dtype=np.float32)
    pred_out = np.asarray(pred_out, dtype=np.float32)
    W_enc = np.asarray(W_enc, dtype=np.float32)
    W_pred = np.asarray(W_pred, dtype=np.float32)
    W_out = np.asarray(W_out, dtype=np.float32)
    b_out = np.asarray(b_out, dtype=np.float32)

    # W_enc.T is [d, j]; chunk both axes by 128 -> [128 dp, KJ, KE, 128 jq]
    # (wenc[p, j, k, q] = W_enc[j*128+q, k*128+p]) so the per-j slice a
    # phase-1 group consumes is one contiguous DMA.
    wenc = np.ascontiguousarray(
        W_enc.T.reshape(KE, 128, KJ, 128).transpose(1, 2, 0, 3)).astype(f16)
    wpred = np.ascontiguousarray(
        W_pred.T.reshape(KE, 128, KJ, 128).transpose(1, 2, 0, 3)).astype(f16)
    # W_out.T is [j, v]; -> [128 jp, VT, KJ, 128 vq] so per-vt slices are
    # contiguous DMAs.
    wout = np.ascontiguousarray(
        W_out.T.reshape(KJ, 128, VT, 128).transpose(1, 2, 0, 3)).astype(f16)
    bias = np.ascontiguousarray(b_out.reshape(VT, 128).T).astype(np.float32)

    in_maps = []
    for b in range(B):
        encb = np.ascontiguousarray(
            enc_out[b].T.reshape(KE, 128, T).transpose(1, 0, 2)).astype(f16)
        predb = np.ascontiguousarray(
            pred_out[b].T.reshape(KE, 128, U).transpose(1, 0, 2)).astype(f16)
        in_maps.append({
            "enc": encb, "pred": predb, "wenc": wenc, "wpred": wpred,
            "wout": wout, "bias": bias,
        })
    return in_maps


def run(inputs, trace=False, **kwargs):
    from concourse.bass_utils import run_bass_kernel_spmd

    _enable_jax_compile_cache()
    nc = _get_compiled()
    in_maps = _prep_inputs(**inputs)
    res = run_bass_kernel_spmd(
        nc, in_maps, core_ids=list(range(N_CORES)), trace=trace, **kwargs)
    out = np.empty((B, T, U, VOCAB), np.float32)
    for b in range(B):
        q = res.results[b]["out"]                   # [VT, 128, U, T] fp16
        # out[b][t, u, vt*128+v] = q[vt, v, u, t]
        out[b] = q.transpose(3, 2, 0, 1).reshape(T, U, VOCAB)
    return out, res


def kernel(**inputs):
    out, _ = run(inputs, trace=False)
    return out
